# revision 6
# baseline (speedup 1.0000x reference)
"""Trainium2 Bass kernel for nn_CrossAttention1D_78640851190158.

Math: k/v in the MHA come from a single cond token broadcast to all T key
positions, so the softmax over identical scores is exactly uniform and the
attention output equals v2 broadcast over T. The whole module collapses to

    out[b, c, t] = x[b, c, t] + y[b, c]
    y[b] = W_eff @ cond[b] + b_eff

where W_eff = proj_w @ out_w @ wv2 @ Wv  (wv2 = in_proj_w[2C:]) and b_eff
folds all the biases through the same chain. The LayerNorm / q path
contributes nothing to the output for ANY input values. The tiny per-batch
vector y (512 floats) is folded on the host along with the weights; the
device does the memory-bound part: stream all of x through SBUF once and
add y broadcast over T (4 MB of HBM traffic per core).

Sharding: pure data parallelism over batch B=8 across the 8 cores.

Device schedule (per core), tuned from ntff traces:
  - Loads (4 x 512 KB, 4 KB row-runs) on the SP HWDGE queue. 4 KB
    contiguous runs matter: the DMA engines cap packets at 4 KB and have a
    fixed ~165-195 ns/packet pitch per engine, so sub-4KB runs cut the
    per-queue rate roughly linearly (2 KB runs -> ~200 GB/s vs ~350-400).
  - y ships transposed as [4, 128] (4 big packets instead of 128x16 B; a
    [128, 4] DMA costs ~2.5 us of queue-head stall at packet pitch) glued
    with an I4 identity; the PE transposes it into PSUM and the adds read
    their per-partition scalar STRAIGHT from PSUM (canonical
    matmul.then_inc -> vector.wait -> read; no drain/copy — the y read on
    the store queue serializes behind the first load chunk, same-direction
    DMAs serialize across queues, so every cycle of post-DMA y latency
    gates the first add and with it the store stream).
  - Adds (tensor_scalar per chunk) on DVE chase the load completions.
  - Stores chase the adds on the ACT HWDGE queue. Loads and stores on
    opposite queues overlap (separate read/write directions); two queues in
    the SAME direction serialize, and mixing directions in one queue
    collapses its rate, so one queue per direction is the fastest shape.
"""

import numpy as np

B, C, T, COND = 8, 512, 1024, 256
N_CORES = 8
P = 128          # SBUF partitions; partition p holds channels 4p..4p+3
NQ = 4           # chunks == channel quarters; chunk h is channel 4p+h
QW = C * T // P // NQ  # 1024 columns per chunk

_cache = {}


def build_kernel():
    import concourse.mybir as mybir
    from concourse import bacc

    f32 = mybir.dt.float32
    # Bacc (not plain Bass): its compile() runs generate_event_semaphores,
    # which splits multi-sem waits to satisfy TRN2's 1-wait-per-instruction
    # constraint.
    nc = bacc.Bacc()

    x_d = nc.dram_tensor("x", [P, NQ * QW], f32, kind="ExternalInput")
    # y_d rows 0..3 = quarter h; cols 0:128 = y^T (col p = y[4p+h]),
    # cols 128:132 = I4 for the PE transpose.
    y_d = nc.dram_tensor("yb", [4, 132], f32, kind="ExternalInput")
    out_d = nc.dram_tensor("out", [P, NQ * QW], f32, kind="ExternalOutput")

    from contextlib import ExitStack
    ctx = ExitStack()
    s_y = ctx.enter_context(nc.semaphore("s_y"))
    s_yt = ctx.enter_context(nc.semaphore("s_yt"))
    s_x = [ctx.enter_context(nc.semaphore(f"s_x{h}")) for h in range(NQ)]
    s_a = ctx.enter_context(nc.semaphore("s_a"))
    s_o = ctx.enter_context(nc.semaphore("s_o"))
    xt = ctx.enter_context(nc.sbuf_tensor("xt", [P, NQ * QW], f32))
    ysb = ctx.enter_context(nc.sbuf_tensor("ysb", [4, 132], f32))
    yp = ctx.enter_context(nc.psum_tensor("yp", [P, 4], f32))

    def chunk(tensor, h):
        return tensor[:, h * QW:(h + 1) * QW]

    # Issue the input streams in the entry basic block, right after the bass
    # preamble barrier — ahead of the Block-entry ceremony.
    nc.scalar.dma_start(out=ysb[:], in_=y_d[:]).then_inc(s_y, 16)
    for h in range(NQ):
        nc.sync.dma_start(out=chunk(xt, h), in_=chunk(x_d, h)).then_inc(s_x[h], 16)

    with nc.Block() as block:
        @block.scalar
        def _(scalar):
            for h in range(NQ):
                scalar.wait_ge(s_a, h + 1)
                scalar.dma_start(out=chunk(out_d, h), in_=chunk(xt, h)).then_inc(s_o, 16)
            scalar.wait_ge(s_o, 16 * NQ)

        @block.tensor
        def _(tensor):
            tensor.wait_ge(s_y, 16)
            tensor.transpose(yp[:], ysb[0:4, 0:128], ysb[0:4, 128:132]).then_inc(s_yt, 1)

        @block.vector
        def _(vector):
            vector.wait_ge(s_yt, 1)
            for h in range(NQ):
                vector.wait_ge(s_x[h], 16)
                vector.tensor_scalar_add(
                    out=chunk(xt, h), in0=chunk(xt, h), scalar1=yp[:, h:h + 1],
                ).then_inc(s_a, 1)

    nc.compile()
    ctx.close()
    return nc


def fold_weights(Wv, bv, in_proj_w, in_proj_b, out_w, out_b, proj_w, proj_b):
    """Fold the v-path weight chain into one [C, COND] map (float64)."""
    wv2 = np.asarray(in_proj_w, np.float64)[2 * C:]
    bv2 = np.asarray(in_proj_b, np.float64)[2 * C:]
    Wv = np.asarray(Wv, np.float64)
    bv = np.asarray(bv, np.float64)
    out_w = np.asarray(out_w, np.float64)
    out_b = np.asarray(out_b, np.float64)
    proj_w = np.asarray(proj_w, np.float64)
    proj_b = np.asarray(proj_b, np.float64)

    po = proj_w @ out_w
    W_eff = po @ wv2 @ Wv
    b_eff = proj_b + proj_w @ out_b + po @ bv2 + po @ wv2 @ bv
    return W_eff, b_eff


def prepare_in_maps(inputs):
    x = np.ascontiguousarray(np.asarray(inputs["x"], np.float32))  # [B, C, T]
    W_eff, b_eff = fold_weights(
        inputs["Wv"], inputs["bv"], inputs["in_proj_w"], inputs["in_proj_b"],
        inputs["out_w"], inputs["out_b"], inputs["proj_w"], inputs["proj_b"],
    )
    cond = np.asarray(inputs["cond"], np.float64)          # [B, COND]
    y = (cond @ W_eff.T + b_eff).astype(np.float32)        # [B, C]

    eye4 = np.eye(4, dtype=np.float32)
    in_maps = []
    for b in range(B):
        yT = np.ascontiguousarray(y[b].reshape(P, 4).T)    # [4, 128]
        yd = np.concatenate([yT, eye4], axis=1)            # [4, 132]
        in_maps.append({
            "x": x[b].reshape(P, NQ * QW),
            "yb": np.ascontiguousarray(yd),
        })
    return in_maps


def kernel(**inputs):
    from concourse.bass_utils import run_bass_kernel_spmd

    if "nc" not in _cache:
        _cache["nc"] = build_kernel()
    nc = _cache["nc"]
    in_maps = prepare_in_maps(inputs)
    res = run_bass_kernel_spmd(nc, in_maps, list(range(N_CORES)))
    out = np.stack([r["out"].reshape(C, T) for r in res.results])
    return out.astype(np.float32)
